# revision 5
# baseline (speedup 1.0000x reference)
"""ComplexGRUCell forward on 8 Trainium2 NeuronCores — Gauss 3M + fp16.

Strategy (data-parallel, feat-major compute):
  - Shard batch B=65536 across 8 cores (8192 rows each).
  - Host-side: transpose x/h slices to [256, 8192] fp16 (feature-major),
    precompute s-streams sx = xr+xi, sh = hr+hi, and combine the complex
    weight pairs of the r/z gates and the x3 candidate path into Gauss
    3-multiplication form:
        P1 = Wr s,  P2 = -(Wr+Wi) xi,  P3 = (Wi-Wr) xr
        re = P1 + P2,  im = P1 + P3
    (3 real matmuls per complex linear instead of 4). The small g3 path
    stays in standard 4-matmul form so its pre-activation lands complete
    in PSUM and the scalar engine can apply the bias directly.
  - Device: PE accumulates in fp16 at 1 cyc/row. P1 banks are drained by
    scalar-engine copies (prompt, keeps PSUM rotation deep); Gauss fixup
    adds (PSUM + SBUF fp16) and the complex elementwise tail run on DVE in
    fp16 (2x mode). Only the final tail (tm, ot) runs on Pool — its window
    overlaps the DVE's PSUM-bound fixup phase, where SBUF contention from
    the Q7 cores doesn't hurt.
  - Host-side: transpose fp16 outputs back to [B, 256] f32 and stack.

Self-contained: hardcodes B=65536, I=H=256, 8 cores.
"""

import numpy as np

import concourse.bass as bass  # noqa: F401
import concourse.mybir as mybir
import concourse.tile as tile
from concourse import bacc, bass_utils

F32 = mybir.dt.float32
F16 = mybir.dt.float16
AF = mybir.ActivationFunctionType
ALU = mybir.AluOpType

B_TOTAL = 65536
N_CORES = 8
B_LOC = B_TOTAL // N_CORES  # 8192
H = 256
NB = 512                    # batch columns per tile
KC = H // 128               # 2 feature chunks (partition dim)

_STREAMS = ["xrT", "xiT", "sxT", "hrT", "hiT", "shT"]
_SI = {s: i for i, s in enumerate(_STREAMS)}

# Gauss paths: (path) -> product -> streams consumed (KC chunks each).
_GAUSS_PATHS = {
    "r":  {"P1": ["sxT", "shT"], "P2": ["xiT", "hiT"], "P3": ["xrT", "hrT"]},
    "z":  {"P1": ["sxT", "shT"], "P2": ["xiT", "hiT"], "P3": ["xrT", "hrT"]},
    "x3": {"P1": ["sxT"], "P2": ["xiT"], "P3": ["xrT"]},
}
# g3 standard form: re consumes [hrT (R3r), hiT (-R3i)], im [hiT (R3r), hrT (R3i)]
_G3_STREAMS = {"re": ["hrT", "hiT"], "im": ["hiT", "hrT"]}

_ACCS = ["r_re", "r_im", "z_re", "z_im", "x3_re", "x3_im", "g3_re", "g3_im"]

# Module-level knobs for the test harness (grading path leaves them alone).
TRACE = False
LAST_RESULT = None
N_TILES = B_LOC // NB       # 16

_CACHED_NC = None


def _build_nc(n_tiles=N_TILES, num_devices=N_CORES):
    b_loc = n_tiles * NB
    nc = bacc.Bacc("TRN2", target_bir_lowering=False, debug=False,
                   num_devices=num_devices)

    ins = {}
    for s in _STREAMS:
        ins[s] = nc.dram_tensor(s, (H, b_loc), F16, kind="ExternalInput")
    for p, prods in _GAUSS_PATHS.items():
        for q, streams in prods.items():
            ncols = len(streams) * KC * 256
            ins[f"w_{p}_{q}"] = nc.dram_tensor(
                f"w_{p}_{q}", (128, ncols), F16, kind="ExternalInput")
    for half in ("re", "im"):
        ins[f"w_g3_{half}"] = nc.dram_tensor(
            f"w_g3_{half}", (128, 2 * KC * 256), F16, kind="ExternalInput")
    ins["biases"] = nc.dram_tensor("biases", (128, 16), F32,
                                   kind="ExternalInput")
    out_r = nc.dram_tensor("outT_r", (H, b_loc), F16, kind="ExternalOutput")
    out_i = nc.dram_tensor("outT_i", (H, b_loc), F16, kind="ExternalOutput")

    bias_col = {}
    for ai, a in enumerate(_ACCS):
        for mo in range(2):
            bias_col[(a, mo)] = ai * 2 + mo

    with tile.TileContext(nc) as tc:
        with (
            tc.tile_pool(name="wpool", bufs=1) as wpool,
            tc.tile_pool(name="mvpool", bufs=2) as mvpool,
            tc.tile_pool(name="spool", bufs=2) as spool,
            tc.tile_pool(name="opool", bufs=3) as opool,
            tc.tile_pool(name="psum", bufs=1, space="PSUM") as psum,
        ):
            # ---- one-time weight/bias loads -------------------------------
            wt = {}

            def load_w(key, ncols):
                t = wpool.tile([128, ncols], F16, name=f"wt_{key}",
                               tag=f"wt_{key}")
                nc.sync.dma_start(t[:], ins[f"w_{key}"][:])
                wt[key] = t

            def w_ap(key, ki, mo):
                t = wt[key]
                return t[:, ki * 256 + mo * 128:ki * 256 + (mo + 1) * 128]

            def load_mv(c0, streams=_STREAMS, mv=None):
                mv = {} if mv is None else mv
                for s in streams:
                    si = _SI[s]
                    nbufs = 3 if s in ("hrT", "hiT") else 2
                    for k in range(KC):
                        m = mvpool.tile([128, NB], F16, name=f"mv{si}{k}",
                                        tag=f"mv{si}{k}", bufs=nbufs)
                        nc.sync.dma_start(
                            m[:], ins[s][k * 128:(k + 1) * 128, c0:c0 + NB])
                        mv[(s, k)] = m
                return mv

            # r-gate weights first so tile-0 matmuls can start early.
            for q in ("P1", "P2", "P3"):
                load_w(f"r_{q}", 2 * KC * 256)
            mv0 = load_mv(0, streams=("sxT", "shT", "xiT", "hiT"))
            for q in ("P1", "P2", "P3"):
                load_w(f"z_{q}", 2 * KC * 256)
            load_mv(0, streams=("xrT", "hrT"), mv=mv0)
            for q in ("P1", "P2", "P3"):
                load_w(f"x3_{q}", KC * 256)
            for half in ("re", "im"):
                load_w(f"g3_{half}", 2 * KC * 256)
            bt = wpool.tile([128, 16], F32, name="bias_t", tag="bias_t")
            nc.sync.dma_start(bt[:], ins["biases"][:])

            def bias_ap(a, mo):
                c = bias_col[(a, mo)]
                return bt[:, c:c + 1]

            # ---- per batch tile -------------------------------------------
            for t_idx in range(n_tiles):
                c0 = t_idx * NB
                mv = mv0 if t_idx == 0 else load_mv(c0)

                for mo in range(2):
                    # PSUM: single-bank tiles (P1s + g3 halves) have prompt
                    # scalar-engine consumers -> shallow rotation is safe.
                    # Double-bank P23 tiles are consumed by DVE fixups ->
                    # give them a 3-deep rotation (6 banks).
                    def ps1_tile(nm):
                        return psum.tile([128, NB], F32, name=f"ps1_{nm}{mo}",
                                         tag="ps1", bufs=2)

                    def ps23_tile(nm):
                        return psum.tile([128, 2 * NB], F32,
                                         name=f"ps23_{nm}{mo}", tag="ps23",
                                         bufs=3)

                    def accum(out_ap, wkey, streams, mo):
                        n_mm = len(streams) * KC
                        j = 0
                        for s in streams:
                            for k in range(KC):
                                ki = (j // KC) * KC + k
                                nc.tensor.matmul(
                                    out_ap, w_ap(wkey, ki, mo), mv[(s, k)][:],
                                    start=(j == 0), stop=(j == n_mm - 1))
                                j += 1

                    pp = {}
                    for p in ("r", "z", "x3"):
                        p1 = ps1_tile(p)
                        p23 = ps23_tile(p)
                        prods = _GAUSS_PATHS[p]
                        accum(p1[:], f"{p}_P1", prods["P1"], mo)
                        accum(p23[:, 0:NB], f"{p}_P2", prods["P2"], mo)
                        accum(p23[:, NB:], f"{p}_P3", prods["P3"], mo)
                        pp[p] = (p1, p23)
                    pg_re = ps1_tile("gre")
                    pg_im = ps1_tile("gim")
                    accum(pg_re[:], "g3_re", _G3_STREAMS["re"], mo)
                    accum(pg_im[:], "g3_im", _G3_STREAMS["im"], mo)

                    # ---- epilogue for this (tile, mo) ----------------------
                    # Gauss fixups: [re|im] = P23 + [P1|P1]. The DVE can only
                    # read ONE input from PSUM, so P1 is first copied to SBUF
                    # fp16 by the scalar engine (prompt PSUM drain + feeds the
                    # DVE a cheap fp16 operand).
                    def fixup(p, out_t):
                        p1, p23 = pp[p]
                        p1sb = spool.tile([128, NB], F16, name=f"p1sb_{p}{mo}",
                                          tag=f"p1sb_{p}")
                        nc.scalar.copy(p1sb[:], p1[:])
                        in0 = p23[:].rearrange("p (a b) -> p a b", a=2)
                        in1 = p1sb[:].unsqueeze(1).broadcast_to([128, 2, NB])
                        out_ap = out_t[:].rearrange("p (a b) -> p a b", a=2)
                        nc.vector.tensor_add(out_ap, in0, in1)

                    rpre = spool.tile([128, 2 * NB], F16, name=f"rpre{mo}",
                                      tag="rpre")
                    zpre = spool.tile([128, 2 * NB], F16, name=f"zpre{mo}",
                                      tag="zpre")
                    x3p = spool.tile([128, 2 * NB], F16, name=f"x3p{mo}",
                                     tag="x3p")
                    g3 = spool.tile([128, 2 * NB], F16, name=f"g3{mo}",
                                    tag="g3")
                    fixup("r", rpre)
                    fixup("z", zpre)
                    fixup("x3", x3p)
                    # g3 pre-act is complete in PSUM: bias via scalar engine.
                    nc.scalar.activation(g3[:, 0:NB], pg_re[:],
                                         AF.Identity,
                                         bias=bias_ap("g3_re", mo))
                    nc.scalar.activation(g3[:, NB:], pg_im[:],
                                         AF.Identity,
                                         bias=bias_ap("g3_im", mo))

                    sr = spool.tile([128, 2 * NB], F16, name=f"sr{mo}",
                                    tag="sr")
                    sz = spool.tile([128, 2 * NB], F16, name=f"sz{mo}",
                                    tag="sz")
                    nc.scalar.activation(sr[:, 0:NB], rpre[:, 0:NB],
                                         AF.Sigmoid, bias=bias_ap("r_re", mo))
                    nc.scalar.activation(sr[:, NB:], rpre[:, NB:],
                                         AF.Sigmoid, bias=bias_ap("r_im", mo))
                    nc.scalar.activation(sz[:, 0:NB], zpre[:, 0:NB],
                                         AF.Sigmoid, bias=bias_ap("z_re", mo))
                    nc.scalar.activation(sz[:, NB:], zpre[:, NB:],
                                         AF.Sigmoid, bias=bias_ap("z_im", mo))

                    def swap_halves(t):
                        return t[:].rearrange("p (a b) -> p a b",
                                              a=2)[:, ::-1, :]

                    def pair(t):
                        return t[:].rearrange("p (a b) -> p a b", a=2)

                    # h3 = r * g3 (complex); ss = x3p + h3
                    u = spool.tile([128, 2 * NB], F16, name=f"u{mo}", tag="u")
                    v = spool.tile([128, 2 * NB], F16, name=f"v{mo}", tag="v")
                    h3 = spool.tile([128, 2 * NB], F16, name=f"h3{mo}",
                                    tag="h3")
                    ss = spool.tile([128, 2 * NB], F16, name=f"ss{mo}",
                                    tag="ss")
                    nn = spool.tile([128, 2 * NB], F16, name=f"nn{mo}",
                                    tag="nn")
                    nc.vector.tensor_mul(u[:], sr[:], g3[:])
                    nc.vector.tensor_mul(pair(v), pair(sr), swap_halves(g3))
                    nc.vector.tensor_sub(h3[:, 0:NB], u[:, 0:NB], u[:, NB:])
                    nc.vector.tensor_add(h3[:, NB:], v[:, 0:NB], v[:, NB:])
                    nc.vector.tensor_add(ss[:], x3p[:], h3[:])
                    nc.scalar.activation(nn[:, 0:NB], ss[:, 0:NB],
                                         AF.Tanh, bias=bias_ap("x3_re", mo))
                    nc.scalar.activation(nn[:, NB:], ss[:, NB:],
                                         AF.Tanh, bias=bias_ap("x3_im", mo))

                    # d = h - n ; out = n + z*d (complex)
                    d = spool.tile([128, 2 * NB], F16, name=f"d{mo}", tag="d")
                    p_ = spool.tile([128, 2 * NB], F16, name=f"p{mo}",
                                    tag="p")
                    q_ = spool.tile([128, 2 * NB], F16, name=f"q{mo}",
                                    tag="q")
                    tm = spool.tile([128, 2 * NB], F16, name=f"tm{mo}",
                                    tag="tm")
                    ot = opool.tile([128, 2 * NB], F16, name=f"ot{mo}",
                                    tag="ot")
                    nc.vector.tensor_sub(d[:, 0:NB], mv[("hrT", mo)][:],
                                         nn[:, 0:NB])
                    nc.vector.tensor_sub(d[:, NB:], mv[("hiT", mo)][:],
                                         nn[:, NB:])
                    nc.vector.tensor_mul(p_[:], sz[:], d[:])
                    nc.vector.tensor_mul(pair(q_), pair(sz), swap_halves(d))
                    # tail on Pool: its window overlaps the DVE fixup phase
                    # of the next (tile, mo), which is PSUM-bound and immune
                    # to the Q7 SBUF contention.
                    nc.gpsimd.tensor_sub(tm[:, 0:NB], p_[:, 0:NB],
                                         p_[:, NB:])
                    nc.gpsimd.tensor_add(tm[:, NB:], q_[:, 0:NB],
                                         q_[:, NB:])
                    nc.gpsimd.tensor_add(ot[:], nn[:], tm[:])

                    nc.sync.dma_start(
                        out_r[mo * 128:(mo + 1) * 128, c0:c0 + NB],
                        ot[:, 0:NB])
                    nc.sync.dma_start(
                        out_i[mo * 128:(mo + 1) * 128, c0:c0 + NB],
                        ot[:, NB:])

    nc.compile()
    return nc


def _prep_weights(p):
    """Host-side weight/bias combination -> device layouts (fp16)."""
    def stk(mats):  # list of [256,256] -> stationary layout [128, n*256]
        W = np.concatenate(mats, axis=1)          # [out=256, in_total]
        WT = np.ascontiguousarray(W.T)            # [in_total, 256]
        n = WT.shape[0] // 128
        return np.ascontiguousarray(
            WT.reshape(n, 128, 256).transpose(1, 0, 2).reshape(128, n * 256)
        ).astype(np.float16)

    def gauss(Wr, Wi):
        return Wr, -(Wr + Wi), (Wi - Wr)

    w = {}
    for path, (wx, wh) in (("r", ("w1", "r1")), ("z", ("w2", "r2"))):
        aW = gauss(p[wx + "Wr"], p[wx + "Wi"])   # x-side linear
        aR = gauss(p[wh + "Wr"], p[wh + "Wi"])   # h-side linear
        for qi, q in enumerate(("P1", "P2", "P3")):
            w[f"w_{path}_{q}"] = stk([aW[qi], aR[qi]])
    aX = gauss(p["w3Wr"], p["w3Wi"])
    for qi, q in enumerate(("P1", "P2", "P3")):
        w[f"w_x3_{q}"] = stk([aX[qi]])
    w["w_g3_re"] = stk([p["r3Wr"], -p["r3Wi"]])
    w["w_g3_im"] = stk([p["r3Wr"], p["r3Wi"]])

    bias = {
        "r_re": p["w1br"] - p["w1bi"] + p["r1br"] - p["r1bi"],
        "r_im": p["w1br"] + p["w1bi"] + p["r1br"] + p["r1bi"],
        "z_re": p["w2br"] - p["w2bi"] + p["r2br"] - p["r2bi"],
        "z_im": p["w2br"] + p["w2bi"] + p["r2br"] + p["r2bi"],
        "x3_re": p["w3br"] - p["w3bi"],
        "x3_im": p["w3br"] + p["w3bi"],
        "g3_re": p["r3br"] - p["r3bi"],
        "g3_im": p["r3br"] + p["r3bi"],
    }
    bcols = np.zeros((128, 16), dtype=np.float32)
    for ai, a in enumerate(_ACCS):
        for mo in range(2):
            bcols[:, ai * 2 + mo] = np.asarray(bias[a])[mo * 128:(mo + 1) * 128]
    w["biases"] = bcols
    return w


def kernel(**inputs):
    global _CACHED_NC, LAST_RESULT
    if _CACHED_NC is None:
        _CACHED_NC = _build_nc()
    nc = _CACHED_NC

    wmaps = _prep_weights(inputs)

    xr = np.asarray(inputs["xr"])
    xi = np.asarray(inputs["xi"])
    hr = np.asarray(inputs["hr"])
    hi = np.asarray(inputs["hi"])

    in_maps = []
    for c in range(N_CORES):
        sl = slice(c * B_LOC, (c + 1) * B_LOC)
        m = dict(wmaps)
        m["xrT"] = np.ascontiguousarray(xr[sl].T).astype(np.float16)
        m["xiT"] = np.ascontiguousarray(xi[sl].T).astype(np.float16)
        m["sxT"] = np.ascontiguousarray((xr[sl] + xi[sl]).T).astype(np.float16)
        m["hrT"] = np.ascontiguousarray(hr[sl].T).astype(np.float16)
        m["hiT"] = np.ascontiguousarray(hi[sl].T).astype(np.float16)
        m["shT"] = np.ascontiguousarray((hr[sl] + hi[sl]).T).astype(np.float16)
        in_maps.append(m)

    kwargs = {}
    if TRACE:
        import sys, types
        try:
            from trn_agent_boot.trn_boot import _ntff_profile_via_ctypes
            mod = types.ModuleType("antenv.axon_hooks")
            mod._hook = _ntff_profile_via_ctypes('/opt/axon/libaxon_pjrt.so')
            mod.get_axon_ntff_profile_hook = lambda: mod._hook
            mod.set_axon_ntff_profile_hook = (
                lambda h: setattr(mod, "_hook", h))
            sys.modules["antenv.axon_hooks"] = mod
            kwargs["trace"] = True
        except Exception:
            pass

    res = bass_utils.run_bass_kernel_spmd(
        nc, in_maps, core_ids=list(range(N_CORES)), **kwargs)
    LAST_RESULT = res

    out = np.empty((2, B_TOTAL, H), dtype=np.float32)
    for c in range(N_CORES):
        sl = slice(c * B_LOC, (c + 1) * B_LOC)
        out[0, sl] = res.results[c]["outT_r"].astype(np.float32).T
        out[1, sl] = res.results[c]["outT_i"].astype(np.float32).T
    return out


# revision 6
# speedup vs baseline: 1.2947x; 1.2947x over previous
"""ComplexGRUCell forward on 8 Trainium2 NeuronCores — Gauss 3M + fp16.

Strategy (data-parallel, feat-major compute):
  - Shard batch B=65536 across 8 cores (8192 rows each).
  - Host-side: transpose x/h slices to [256, 8192] fp16 (feature-major),
    precompute s-streams sx = xr+xi, sh = hr+hi, and combine the complex
    weight pairs of the r/z gates and the x3 candidate path into Gauss
    3-multiplication form:
        P1 = Wr s,  P2 = -(Wr+Wi) xi,  P3 = (Wi-Wr) xr
        re = P1 + P2,  im = P1 + P3
    (3 real matmuls per complex linear instead of 4). The small g3 path
    stays in standard 4-matmul form so its pre-activation lands complete
    in PSUM and the scalar engine can apply the bias directly.
  - Device: PE accumulates in fp16 at 1 cyc/row. P1 banks are drained by
    scalar-engine copies (prompt, keeps PSUM rotation deep); Gauss fixup
    adds (PSUM + SBUF fp16) and the complex elementwise tail run on DVE in
    fp16 (2x mode). Only the final tail (tm, ot) runs on Pool — its window
    overlaps the DVE's PSUM-bound fixup phase, where SBUF contention from
    the Q7 cores doesn't hurt.
  - Host-side: transpose fp16 outputs back to [B, 256] f32 and stack.

Self-contained: hardcodes B=65536, I=H=256, 8 cores.
"""

import numpy as np

import concourse.bass as bass  # noqa: F401
import concourse.mybir as mybir
import concourse.tile as tile
from concourse import bacc, bass_utils

F32 = mybir.dt.float32
F16 = mybir.dt.float16
AF = mybir.ActivationFunctionType
ALU = mybir.AluOpType

B_TOTAL = 65536
N_CORES = 8
B_LOC = B_TOTAL // N_CORES  # 8192
H = 256
NB = 512                    # batch columns per tile
KC = H // 128               # 2 feature chunks (partition dim)

_STREAMS = ["xrT", "xiT", "sxT", "hrT", "hiT", "shT"]
_SI = {s: i for i, s in enumerate(_STREAMS)}

# Gauss paths: (path) -> product -> streams consumed (KC chunks each).
_GAUSS_PATHS = {
    "r":  {"P1": ["sxT", "shT"], "P2": ["xiT", "hiT"], "P3": ["xrT", "hrT"]},
    "z":  {"P1": ["sxT", "shT"], "P2": ["xiT", "hiT"], "P3": ["xrT", "hrT"]},
    "x3": {"P1": ["sxT"], "P2": ["xiT"], "P3": ["xrT"]},
}
# g3 standard form: re consumes [hrT (R3r), hiT (-R3i)], im [hiT (R3r), hrT (R3i)]
_G3_STREAMS = {"re": ["hrT", "hiT"], "im": ["hiT", "hrT"]}

_ACCS = ["r_re", "r_im", "z_re", "z_im", "x3_re", "x3_im", "g3_re", "g3_im"]

# Module-level knobs for the test harness (grading path leaves them alone).
TRACE = False
LAST_RESULT = None
N_TILES = B_LOC // NB       # 16

_CACHED_NC = None


def _build_nc(n_tiles=N_TILES, num_devices=N_CORES):
    b_loc = n_tiles * NB
    nc = bacc.Bacc("TRN2", target_bir_lowering=False, debug=False,
                   num_devices=num_devices)

    ins = {}
    for s in _STREAMS:
        ins[s] = nc.dram_tensor(s, (H, b_loc), F16, kind="ExternalInput")
    for p, prods in _GAUSS_PATHS.items():
        for q, streams in prods.items():
            ncols = len(streams) * KC * 256
            ins[f"w_{p}_{q}"] = nc.dram_tensor(
                f"w_{p}_{q}", (128, ncols), F16, kind="ExternalInput")
    for half in ("re", "im"):
        ins[f"w_g3_{half}"] = nc.dram_tensor(
            f"w_g3_{half}", (128, 2 * KC * 256), F16, kind="ExternalInput")
    ins["biases"] = nc.dram_tensor("biases", (128, 16), F32,
                                   kind="ExternalInput")
    ins["ident"] = nc.dram_tensor("ident", (128, 128), F16,
                                  kind="ExternalInput")
    out_r = nc.dram_tensor("outT_r", (H, b_loc), F16, kind="ExternalOutput")
    out_i = nc.dram_tensor("outT_i", (H, b_loc), F16, kind="ExternalOutput")

    bias_col = {}
    for ai, a in enumerate(_ACCS):
        for mo in range(2):
            bias_col[(a, mo)] = ai * 2 + mo

    with tile.TileContext(nc) as tc:
        with (
            tc.tile_pool(name="wpool", bufs=1) as wpool,
            tc.tile_pool(name="mvpool", bufs=2) as mvpool,
            tc.tile_pool(name="spool", bufs=2) as spool,
            tc.tile_pool(name="opool", bufs=3) as opool,
            tc.tile_pool(name="psum", bufs=1, space="PSUM") as psum,
        ):
            # ---- one-time weight/bias loads -------------------------------
            wt = {}

            def load_w(key, ncols):
                t = wpool.tile([128, ncols], F16, name=f"wt_{key}",
                               tag=f"wt_{key}")
                nc.sync.dma_start(t[:], ins[f"w_{key}"][:])
                wt[key] = t

            def w_ap(key, ki, mo):
                t = wt[key]
                return t[:, ki * 256 + mo * 128:ki * 256 + (mo + 1) * 128]

            def load_mv(c0, streams=_STREAMS, mv=None):
                mv = {} if mv is None else mv
                for s in streams:
                    si = _SI[s]
                    nbufs = 3 if s in ("hrT", "hiT") else 2
                    for k in range(KC):
                        m = mvpool.tile([128, NB], F16, name=f"mv{si}{k}",
                                        tag=f"mv{si}{k}", bufs=nbufs)
                        nc.sync.dma_start(
                            m[:], ins[s][k * 128:(k + 1) * 128, c0:c0 + NB])
                        mv[(s, k)] = m
                return mv

            # r-gate weights first so tile-0 matmuls can start early.
            for q in ("P1", "P2", "P3"):
                load_w(f"r_{q}", 2 * KC * 256)
            mv0 = load_mv(0, streams=("sxT", "shT", "xiT", "hiT"))
            for q in ("P1", "P2", "P3"):
                load_w(f"z_{q}", 2 * KC * 256)
            load_mv(0, streams=("xrT", "hrT"), mv=mv0)
            for q in ("P1", "P2", "P3"):
                load_w(f"x3_{q}", KC * 256)
            for half in ("re", "im"):
                load_w(f"g3_{half}", 2 * KC * 256)
            bt = wpool.tile([128, 16], F32, name="bias_t", tag="bias_t")
            nc.sync.dma_start(bt[:], ins["biases"][:])
            id_t = wpool.tile([128, 128], F16, name="ident_t", tag="ident_t")
            nc.sync.dma_start(id_t[:], ins["ident"][:])

            def bias_ap(a, mo):
                c = bias_col[(a, mo)]
                return bt[:, c:c + 1]

            # ---- per batch tile -------------------------------------------
            for t_idx in range(n_tiles):
                c0 = t_idx * NB
                mv = mv0 if t_idx == 0 else load_mv(c0)

                for mo in range(2):
                    # PSUM: single-bank tiles (P1s + g3 halves) have prompt
                    # scalar-engine consumers -> shallow rotation is safe.
                    # Double-bank P23 tiles are consumed by DVE fixups ->
                    # give them a 3-deep rotation (6 banks).
                    def ps1_tile(nm):
                        return psum.tile([128, NB], F32, name=f"ps1_{nm}{mo}",
                                         tag="ps1", bufs=2)

                    def ps23_tile(nm):
                        return psum.tile([128, 2 * NB], F32,
                                         name=f"ps23_{nm}{mo}", tag="ps23",
                                         bufs=3)

                    def accum(out_ap, wkey, streams, mo, stop=True):
                        n_mm = len(streams) * KC
                        j = 0
                        for s in streams:
                            for k in range(KC):
                                ki = (j // KC) * KC + k
                                nc.tensor.matmul(
                                    out_ap, w_ap(wkey, ki, mo), mv[(s, k)][:],
                                    start=(j == 0),
                                    stop=(stop and j == n_mm - 1))
                                j += 1

                    # Alternate the r-gate P1 strategy per (tile, mo) to
                    # balance PE vs DVE: identity-add folds P1 into the P23
                    # accumulation on the PE (+2 matmuls, no DVE fixup);
                    # otherwise the DVE fixup handles it.
                    r_ident = (t_idx * 2 + mo) % 2 == 0

                    pp = {}
                    p1sb_r = None
                    for p in ("r", "z", "x3"):
                        p1 = ps1_tile(p)
                        p23 = ps23_tile(p)
                        prods = _GAUSS_PATHS[p]
                        accum(p1[:], f"{p}_P1", prods["P1"], mo)
                        if p == "r" and r_ident:
                            p1sb_r = spool.tile([128, NB], F16,
                                                name=f"p1sb_r{mo}",
                                                tag="p1sb_r")
                            nc.scalar.copy(p1sb_r[:], p1[:])
                            accum(p23[:, 0:NB], f"{p}_P2", prods["P2"], mo,
                                  stop=False)
                            nc.tensor.matmul(p23[:, 0:NB], id_t[:],
                                             p1sb_r[:], start=False,
                                             stop=True)
                            accum(p23[:, NB:], f"{p}_P3", prods["P3"], mo,
                                  stop=False)
                            nc.tensor.matmul(p23[:, NB:], id_t[:],
                                             p1sb_r[:], start=False,
                                             stop=True)
                        else:
                            accum(p23[:, 0:NB], f"{p}_P2", prods["P2"], mo)
                            accum(p23[:, NB:], f"{p}_P3", prods["P3"], mo)
                        pp[p] = (p1, p23)
                    pg_re = ps1_tile("gre")
                    pg_im = ps1_tile("gim")
                    accum(pg_re[:], "g3_re", _G3_STREAMS["re"], mo)
                    accum(pg_im[:], "g3_im", _G3_STREAMS["im"], mo)

                    # ---- epilogue for this (tile, mo) ----------------------
                    # Gauss fixups: [re|im] = P23 + [P1|P1]. The DVE can only
                    # read ONE input from PSUM, so P1 is first copied to SBUF
                    # fp16 by the scalar engine (prompt PSUM drain + feeds the
                    # DVE a cheap fp16 operand).
                    def fixup(p, out_t):
                        p1, p23 = pp[p]
                        p1sb = spool.tile([128, NB], F16, name=f"p1sb_{p}{mo}",
                                          tag=f"p1sb_{p}")
                        nc.scalar.copy(p1sb[:], p1[:])
                        in0 = p23[:].rearrange("p (a b) -> p a b", a=2)
                        in1 = p1sb[:].unsqueeze(1).broadcast_to([128, 2, NB])
                        out_ap = out_t[:].rearrange("p (a b) -> p a b", a=2)
                        nc.vector.tensor_add(out_ap, in0, in1)

                    zpre = spool.tile([128, 2 * NB], F16, name=f"zpre{mo}",
                                      tag="zpre")
                    x3p = spool.tile([128, 2 * NB], F16, name=f"x3p{mo}",
                                     tag="x3p")
                    g3 = spool.tile([128, 2 * NB], F16, name=f"g3{mo}",
                                    tag="g3")
                    sr = spool.tile([128, 2 * NB], F16, name=f"sr{mo}",
                                    tag="sr")
                    sz = spool.tile([128, 2 * NB], F16, name=f"sz{mo}",
                                    tag="sz")
                    if r_ident:
                        # r pre-act is complete in PSUM
                        p23r = pp["r"][1]
                        nc.scalar.activation(sr[:, 0:NB], p23r[:, 0:NB],
                                             AF.Sigmoid,
                                             bias=bias_ap("r_re", mo))
                        nc.scalar.activation(sr[:, NB:], p23r[:, NB:],
                                             AF.Sigmoid,
                                             bias=bias_ap("r_im", mo))
                    else:
                        rpre = spool.tile([128, 2 * NB], F16, name=f"rpre{mo}",
                                          tag="rpre")
                        fixup("r", rpre)
                        nc.scalar.activation(sr[:, 0:NB], rpre[:, 0:NB],
                                             AF.Sigmoid,
                                             bias=bias_ap("r_re", mo))
                        nc.scalar.activation(sr[:, NB:], rpre[:, NB:],
                                             AF.Sigmoid,
                                             bias=bias_ap("r_im", mo))
                    fixup("z", zpre)
                    fixup("x3", x3p)
                    # g3 pre-act is complete in PSUM: bias via scalar engine.
                    nc.scalar.activation(g3[:, 0:NB], pg_re[:],
                                         AF.Identity,
                                         bias=bias_ap("g3_re", mo))
                    nc.scalar.activation(g3[:, NB:], pg_im[:],
                                         AF.Identity,
                                         bias=bias_ap("g3_im", mo))

                    nc.scalar.activation(sz[:, 0:NB], zpre[:, 0:NB],
                                         AF.Sigmoid, bias=bias_ap("z_re", mo))
                    nc.scalar.activation(sz[:, NB:], zpre[:, NB:],
                                         AF.Sigmoid, bias=bias_ap("z_im", mo))

                    def swap_halves(t):
                        return t[:].rearrange("p (a b) -> p a b",
                                              a=2)[:, ::-1, :]

                    def pair(t):
                        return t[:].rearrange("p (a b) -> p a b", a=2)

                    # h3 = r * g3 (complex); ss = x3p + h3
                    u = spool.tile([128, 2 * NB], F16, name=f"u{mo}", tag="u")
                    v = spool.tile([128, 2 * NB], F16, name=f"v{mo}", tag="v")
                    h3 = spool.tile([128, 2 * NB], F16, name=f"h3{mo}",
                                    tag="h3")
                    ss = spool.tile([128, 2 * NB], F16, name=f"ss{mo}",
                                    tag="ss")
                    nn = spool.tile([128, 2 * NB], F16, name=f"nn{mo}",
                                    tag="nn")
                    nc.vector.tensor_mul(u[:], sr[:], g3[:])
                    nc.vector.tensor_mul(pair(v), pair(sr), swap_halves(g3))
                    nc.vector.tensor_sub(h3[:, 0:NB], u[:, 0:NB], u[:, NB:])
                    nc.vector.tensor_add(h3[:, NB:], v[:, 0:NB], v[:, NB:])
                    nc.vector.tensor_add(ss[:], x3p[:], h3[:])
                    nc.scalar.activation(nn[:, 0:NB], ss[:, 0:NB],
                                         AF.Tanh, bias=bias_ap("x3_re", mo))
                    nc.scalar.activation(nn[:, NB:], ss[:, NB:],
                                         AF.Tanh, bias=bias_ap("x3_im", mo))

                    # d = h - n ; out = n + z*d (complex), all on DVE
                    d = spool.tile([128, 2 * NB], F16, name=f"d{mo}", tag="d")
                    p_ = spool.tile([128, 2 * NB], F16, name=f"p{mo}",
                                    tag="p")
                    q_ = spool.tile([128, 2 * NB], F16, name=f"q{mo}",
                                    tag="q")
                    tm = spool.tile([128, 2 * NB], F16, name=f"tm{mo}",
                                    tag="tm")
                    ot = opool.tile([128, 2 * NB], F16, name=f"ot{mo}",
                                    tag="ot")
                    nc.vector.tensor_sub(d[:, 0:NB], mv[("hrT", mo)][:],
                                         nn[:, 0:NB])
                    nc.vector.tensor_sub(d[:, NB:], mv[("hiT", mo)][:],
                                         nn[:, NB:])
                    nc.vector.tensor_mul(p_[:], sz[:], d[:])
                    nc.vector.tensor_mul(pair(q_), pair(sz), swap_halves(d))
                    nc.vector.tensor_sub(tm[:, 0:NB], p_[:, 0:NB],
                                         p_[:, NB:])
                    nc.vector.tensor_add(tm[:, NB:], q_[:, 0:NB],
                                         q_[:, NB:])
                    nc.vector.tensor_add(ot[:], nn[:], tm[:])

                    nc.sync.dma_start(
                        out_r[mo * 128:(mo + 1) * 128, c0:c0 + NB],
                        ot[:, 0:NB])
                    nc.sync.dma_start(
                        out_i[mo * 128:(mo + 1) * 128, c0:c0 + NB],
                        ot[:, NB:])

    nc.compile()
    return nc


def _prep_weights(p):
    """Host-side weight/bias combination -> device layouts (fp16)."""
    def stk(mats):  # list of [256,256] -> stationary layout [128, n*256]
        W = np.concatenate(mats, axis=1)          # [out=256, in_total]
        WT = np.ascontiguousarray(W.T)            # [in_total, 256]
        n = WT.shape[0] // 128
        return np.ascontiguousarray(
            WT.reshape(n, 128, 256).transpose(1, 0, 2).reshape(128, n * 256)
        ).astype(np.float16)

    def gauss(Wr, Wi):
        return Wr, -(Wr + Wi), (Wi - Wr)

    w = {}
    for path, (wx, wh) in (("r", ("w1", "r1")), ("z", ("w2", "r2"))):
        aW = gauss(p[wx + "Wr"], p[wx + "Wi"])   # x-side linear
        aR = gauss(p[wh + "Wr"], p[wh + "Wi"])   # h-side linear
        for qi, q in enumerate(("P1", "P2", "P3")):
            w[f"w_{path}_{q}"] = stk([aW[qi], aR[qi]])
    aX = gauss(p["w3Wr"], p["w3Wi"])
    for qi, q in enumerate(("P1", "P2", "P3")):
        w[f"w_x3_{q}"] = stk([aX[qi]])
    w["w_g3_re"] = stk([p["r3Wr"], -p["r3Wi"]])
    w["w_g3_im"] = stk([p["r3Wr"], p["r3Wi"]])

    bias = {
        "r_re": p["w1br"] - p["w1bi"] + p["r1br"] - p["r1bi"],
        "r_im": p["w1br"] + p["w1bi"] + p["r1br"] + p["r1bi"],
        "z_re": p["w2br"] - p["w2bi"] + p["r2br"] - p["r2bi"],
        "z_im": p["w2br"] + p["w2bi"] + p["r2br"] + p["r2bi"],
        "x3_re": p["w3br"] - p["w3bi"],
        "x3_im": p["w3br"] + p["w3bi"],
        "g3_re": p["r3br"] - p["r3bi"],
        "g3_im": p["r3br"] + p["r3bi"],
    }
    bcols = np.zeros((128, 16), dtype=np.float32)
    for ai, a in enumerate(_ACCS):
        for mo in range(2):
            bcols[:, ai * 2 + mo] = np.asarray(bias[a])[mo * 128:(mo + 1) * 128]
    w["biases"] = bcols
    w["ident"] = np.eye(128, dtype=np.float16)
    return w


def kernel(**inputs):
    global _CACHED_NC, LAST_RESULT
    if _CACHED_NC is None:
        _CACHED_NC = _build_nc()
    nc = _CACHED_NC

    wmaps = _prep_weights(inputs)

    xr = np.asarray(inputs["xr"])
    xi = np.asarray(inputs["xi"])
    hr = np.asarray(inputs["hr"])
    hi = np.asarray(inputs["hi"])

    in_maps = []
    for c in range(N_CORES):
        sl = slice(c * B_LOC, (c + 1) * B_LOC)
        m = dict(wmaps)
        m["xrT"] = np.ascontiguousarray(xr[sl].T).astype(np.float16)
        m["xiT"] = np.ascontiguousarray(xi[sl].T).astype(np.float16)
        m["sxT"] = np.ascontiguousarray((xr[sl] + xi[sl]).T).astype(np.float16)
        m["hrT"] = np.ascontiguousarray(hr[sl].T).astype(np.float16)
        m["hiT"] = np.ascontiguousarray(hi[sl].T).astype(np.float16)
        m["shT"] = np.ascontiguousarray((hr[sl] + hi[sl]).T).astype(np.float16)
        in_maps.append(m)

    kwargs = {}
    if TRACE:
        import sys, types
        try:
            from trn_agent_boot.trn_boot import _ntff_profile_via_ctypes
            mod = types.ModuleType("antenv.axon_hooks")
            mod._hook = _ntff_profile_via_ctypes('/opt/axon/libaxon_pjrt.so')
            mod.get_axon_ntff_profile_hook = lambda: mod._hook
            mod.set_axon_ntff_profile_hook = (
                lambda h: setattr(mod, "_hook", h))
            sys.modules["antenv.axon_hooks"] = mod
            kwargs["trace"] = True
        except Exception:
            pass

    res = bass_utils.run_bass_kernel_spmd(
        nc, in_maps, core_ids=list(range(N_CORES)), **kwargs)
    LAST_RESULT = res

    out = np.empty((2, B_TOTAL, H), dtype=np.float32)
    for c in range(N_CORES):
        sl = slice(c * B_LOC, (c + 1) * B_LOC)
        out[0, sl] = res.results[c]["outT_r"].astype(np.float32).T
        out[1, sl] = res.results[c]["outT_i"].astype(np.float32).T
    return out
